# revision 10
# baseline (speedup 1.0000x reference)
"""Trainium2 Bass kernel for 2-layer GCN (GCNConv -> relu -> GCNConv -> Linear).

Strategy (8 NeuronCores, SPMD):
  - Nodes padded to NPAD=100352 and dealt (serpentine, by degree) into 784
    blocks of 128 slots; 98 blocks per core.  Each core owns the aggregation
    for its 98 blocks (edge partition by destination).
  - norm factorizes: out = dinv[dst] * sum_e (h*dinv)[src] (self loops are
    plain edges).  Tables T1=(x@W1)*dinv (bf16) are computed replicated on
    every core; messages are fetched with dma_gather (int16 indices into 4
    row-quarters of the table) and segment-summed via a selection-matrix
    matmul accumulated in PSUM.
  - Layer-2 table T2=(relu(agg1*dinv+b1)@W2)*dinv is exchanged with an
    AllGather, then the same gather/matmul pass produces h2, and the final
    y = h2@Wfc + bfc is written per block.  Host un-permutes rows.
"""

import numpy as np
import ml_dtypes

P = 128
NCORES = 8
NQ = 4
IN_C, HID, OUT_C = 128, 128, 64
CALL = 1024              # rows per dma_gather call (SWDGE packet limit)


def _set_size(n_nodes, bpc):
    """Set problem size (module-global); mini sizes used by the CoreSim check."""
    global N, BPC, NBINS, NPAD, SHARD, QROWS
    N = n_nodes
    BPC = bpc
    NBINS = NCORES * BPC
    NPAD = NBINS * P
    SHARD = BPC * P
    QROWS = NPAD // NQ
    assert N <= NPAD and QROWS <= 32768


_set_size(100000, 98)

_kernel_cache = {}


def _wrap_idx(st):
    """[C, NQ, SLEN] int16 -> [C, NQ, 128, SLEN//16] wrapped+replicated."""
    C, Q, SLEN = st.shape
    w = st.reshape(C, Q, SLEN // 16, 16)
    w = np.swapaxes(w, 2, 3)                       # [C, Q, 16, SLEN//16]
    return np.ascontiguousarray(np.tile(w, (1, 1, 8, 1)))


def _edge_arrays(q, lidx, core, b, dslot):
    """Build per-core gather-index streams and dst-slot arrays for one layer.

    Streams are grouped by (core, quarter, block); each (block, quarter) cell
    is padded to a multiple of 128 lanes (dummy idx 0, dst-slot 255 -> zero
    row in the selection matrix)."""
    lidx = lidx.astype(np.int16)
    cell = (core * NQ + q) * BPC + b
    ncell = NCORES * NQ * BPC
    counts = np.bincount(cell, minlength=ncell)
    K = int(np.ceil(counts.max() / P))
    CAP = K * P
    order = np.argsort(cell, kind="stable")
    start = np.zeros(ncell + 1, np.int64)
    np.cumsum(counts, out=start[1:])
    rank = np.arange(cell.shape[0]) - start[cell[order]]
    pos = cell[order] * CAP + rank
    idx_arr = np.zeros(ncell * CAP, np.int16)
    dl_arr = np.full(ncell * CAP, 255.0, np.float32)
    idx_arr[pos] = lidx[order]
    dl_arr[pos] = dslot[order]
    # pad each (core, quarter) stream to a multiple of CALL so every
    # dma_gather call is a uniform CALL indices (a trailing partial call
    # was observed to wedge the device)
    slen = BPC * CAP
    slen_pad = -(-slen // CALL) * CALL
    st = np.zeros((NCORES, NQ, slen_pad), np.int16)
    st[:, :, :slen] = idx_arr.reshape(NCORES, NQ, slen)
    gidx = _wrap_idx(st)
    dl = dl_arr.reshape(NCORES, NQ, BPC, K, P)
    dl = dl.transpose(0, 4, 2, 1, 3).reshape(NCORES, P, BPC * NQ * K)
    return K, gidx, dl.astype(ml_dtypes.bfloat16)


def _preprocess(x, edge_index, W1, b1, W2, b2, Wfc, bfc):
    src = np.asarray(edge_index[0], dtype=np.int64)
    dst = np.asarray(edge_index[1], dtype=np.int64)
    deg = (np.bincount(dst, minlength=N) + 1).astype(np.float32)
    dinv_pad = np.ones(NPAD, np.float32)
    dinv_pad[:N] = (1.0 / np.sqrt(deg)).astype(np.float32)

    loop = np.arange(N, dtype=np.int64)
    src_a = np.concatenate([src, loop])
    dst_a = np.concatenate([dst, loop])

    # serpentine deal by degree -> (bin, slot); balances per-block edge counts
    key = np.zeros(NPAD, np.float32)
    key[:N] = deg
    order = np.argsort(-key, kind="stable")
    i = np.arange(NPAD)
    r, c = i // NBINS, i % NBINS
    bins_for_rank = np.where(r % 2 == 0, c, NBINS - 1 - c)
    perm_bin = np.empty(NPAD, np.int64)
    perm_slot = np.empty(NPAD, np.int64)
    perm_bin[order] = bins_for_rank
    perm_slot[order] = r
    perm_pos = perm_bin * P + perm_slot
    pos2node = np.empty(NPAD, np.int64)
    pos2node[perm_pos] = np.arange(NPAD)

    ecore = perm_bin[dst_a] // BPC
    eb = perm_bin[dst_a] % BPC
    edslot = perm_slot[dst_a].astype(np.float32)

    # layer 1: subtables are contiguous node-id quarters of the (node-order)
    # T1 table.  layer 2: subtables split by slot//32 within each block so a
    # block's 128 self loops spread 32 per quarter (and the on-device u2
    # write is a contiguous partition range per subtable).
    K1, gidx1, dl1 = _edge_arrays(src_a // QROWS, src_a % QROWS, ecore, eb, edslot)
    spos = perm_pos[src_a]
    sc, sb_, ss = spos // SHARD, (spos % SHARD) // P, spos % P
    q2 = ss // (P // NQ)
    idx2 = sc * (SHARD // NQ) + sb_ * (P // NQ) + (ss % (P // NQ))
    K2, gidx2, dl2 = _edge_arrays(q2, idx2, ecore, eb, edslot)

    xpad = np.zeros((NPAD, IN_C), np.float32)
    xpad[:N] = np.asarray(x, np.float32)
    xT = np.ascontiguousarray(xpad.T).astype(ml_dtypes.bfloat16)  # [128, NPAD]

    dinv_node = np.ascontiguousarray(dinv_pad.reshape(NBINS, P).T)  # [128,784]
    dinv_pos = dinv_pad[pos2node]                            # dinv by position
    dinv_blk = dinv_pos.reshape(NCORES, BPC, P).transpose(0, 2, 1)  # [C,128,98]

    common = {
        "xT": xT,
        "W1": np.asarray(W1, np.float32).astype(ml_dtypes.bfloat16),
        "W2": np.asarray(W2, np.float32),
        "Wfc": np.asarray(Wfc, np.float32),
        "b1x": np.tile(np.asarray(b1, np.float32)[None, :], (P, 1)),
        "b2x": np.tile(np.asarray(b2, np.float32)[None, :], (P, 1)),
        "bfcx": np.tile(np.asarray(bfc, np.float32)[None, :], (P, 1)),
        "iota": np.tile(np.arange(P, dtype=np.float32)[None, :], (P, 1)).astype(
            ml_dtypes.bfloat16
        ),
        "dinv_node": dinv_node,
    }
    in_maps = []
    for c in range(NCORES):
        m = dict(common)
        m["dinv_blk"] = np.ascontiguousarray(dinv_blk[c])
        m["dstloc1"] = np.ascontiguousarray(dl1[c])
        m["dstloc2"] = np.ascontiguousarray(dl2[c])
        m["gidx1"] = np.ascontiguousarray(gidx1[c])
        m["gidx2"] = np.ascontiguousarray(gidx2[c])
        in_maps.append(m)
    return K1, K2, in_maps, perm_pos


def _build(K1, K2):
    import os
    import concourse.bass as bass  # noqa: F401
    import concourse.mybir as mybir
    import concourse.tile as tile
    from concourse import bacc
    from concourse.masks import make_identity

    stop_after = os.environ.get("KB_STOP_AFTER", "")   # "", "A", "B", "CC"
    dt = mybir.dt
    OP = mybir.AluOpType
    _pad = lambda s: -(-s // CALL) * CALL
    SLEN1, SLEN2 = _pad(BPC * K1 * P), _pad(BPC * K2 * P)
    CPP = CALL // P     # chunks per gather call

    nc = bacc.Bacc("TRN2", num_devices=NCORES, target_bir_lowering=False, debug=False,
                   num_swdge_queues=4)

    xT = nc.dram_tensor("xT", [P, NPAD], dt.bfloat16, kind="ExternalInput")
    W1 = nc.dram_tensor("W1", [IN_C, HID], dt.bfloat16, kind="ExternalInput")
    W2 = nc.dram_tensor("W2", [HID, OUT_C], dt.float32, kind="ExternalInput")
    Wfc = nc.dram_tensor("Wfc", [OUT_C, OUT_C], dt.float32, kind="ExternalInput")
    b1x = nc.dram_tensor("b1x", [P, HID], dt.float32, kind="ExternalInput")
    b2x = nc.dram_tensor("b2x", [P, OUT_C], dt.float32, kind="ExternalInput")
    bfcx = nc.dram_tensor("bfcx", [P, OUT_C], dt.float32, kind="ExternalInput")
    iota = nc.dram_tensor("iota", [P, P], dt.bfloat16, kind="ExternalInput")
    dinv_node = nc.dram_tensor("dinv_node", [P, NBINS], dt.float32, kind="ExternalInput")
    dinv_blk = nc.dram_tensor("dinv_blk", [P, BPC], dt.float32, kind="ExternalInput")
    dstloc1 = nc.dram_tensor("dstloc1", [P, BPC * NQ * K1], dt.bfloat16, kind="ExternalInput")
    dstloc2 = nc.dram_tensor("dstloc2", [P, BPC * NQ * K2], dt.bfloat16, kind="ExternalInput")
    gidx1 = nc.dram_tensor("gidx1", [NQ, P, SLEN1 // 16], dt.int16, kind="ExternalInput")
    gidx2 = nc.dram_tensor("gidx2", [NQ, P, SLEN2 // 16], dt.int16, kind="ExternalInput")
    y = nc.dram_tensor("y", [SHARD, OUT_C], dt.float32, kind="ExternalOutput")

    T1 = nc.dram_tensor("T1", [NPAD, HID], dt.bfloat16)
    u2loc = nc.dram_tensor("u2loc", [NQ, SHARD // NQ, P], dt.bfloat16)
    T2q = [nc.dram_tensor(f"T2_{q}", [QROWS, P], dt.bfloat16, addr_space="Shared")
           for q in range(NQ)]

    def agg_pass(sb_g, sb_s, ps, iota_t, tables, Kc, gidx_t, dstloc_ap, elem, nout, epilogue, nblocks=BPC):
        """For each of BPC blocks: gather messages (dma_gather per 1024 rows,
        per quarter), build the selection matrix, matmul-accumulate in PSUM,
        then run the epilogue."""
        slen = _pad(BPC * Kc * P)
        ncalls = slen // CALL
        gtiles = {}
        issued = [0] * NQ

        def issue(q, call):
            n = min(CALL, slen - call * CALL)
            gt = sb_g.tile([P, CPP, elem], dt.bfloat16, tag="gbuf")
            nc.gpsimd.dma_gather(
                out_ap=gt[:, : n // P, :],
                in_ap=tables[q],
                idxs_ap=gidx_t[q][:, call * (CALL // 16) : call * (CALL // 16) + n // 16],
                num_idxs=n,
                num_idxs_reg=n,
                elem_size=elem,
                queue_num=q,
            )
            gtiles[(q, call)] = gt

        for b in range(nblocks):
            # issue one block ahead so transfers overlap this block's compute
            last_call = ((min(b + 2, nblocks) * Kc) - 1) // CPP
            for q in range(NQ):
                while issued[q] <= last_call and issued[q] < ncalls:
                    issue(q, issued[q])
                    issued[q] += 1
            s_all = sb_s.tile([P, NQ * Kc, P], dt.bfloat16, tag="sall")
            nc.vector.tensor_tensor(
                out=s_all[:],
                in0=dstloc_ap[:, b * NQ * Kc : (b + 1) * NQ * Kc].to_broadcast(
                    [P, NQ * Kc, P]
                ),
                in1=iota_t[:].rearrange("p (a b) -> p a b", a=1).to_broadcast(
                    [P, NQ * Kc, P]
                ),
                op=OP.is_equal,
            )
            psum_agg = ps.tile([P, nout], dt.float32, space="PSUM", tag="agg")
            nmm = NQ * Kc
            k = 0
            for q in range(NQ):
                for j in range(Kc):
                    g = b * Kc + j
                    gt = gtiles[(q, g // CPP)]
                    nc.tensor.matmul(
                        out=psum_agg[:],
                        lhsT=s_all[:, q * Kc + j, :],
                        rhs=gt[:, g % CPP, :nout],
                        start=(k == 0),
                        stop=(k == nmm - 1),
                    )
                    k += 1
            epilogue(b, psum_agg)

    with tile.TileContext(nc) as tc:
        with tc.tile_pool(name="const", bufs=1) as cp:
            gconst = {}
            for name, t, shape, dtt in [
                ("W1", W1, [IN_C, HID], dt.bfloat16),
                ("W2", W2, [HID, OUT_C], dt.float32),
                ("Wfc", Wfc, [OUT_C, OUT_C], dt.float32),
                ("b1x", b1x, [P, HID], dt.float32),
                ("b2x", b2x, [P, OUT_C], dt.float32),
                ("bfcx", bfcx, [P, OUT_C], dt.float32),
                ("iota", iota, [P, P], dt.bfloat16),
                ("dinv_node", dinv_node, [P, NBINS], dt.float32),
                ("dinv_blk", dinv_blk, [P, BPC], dt.float32),
                ("dstloc1", dstloc1, [P, BPC * NQ * K1], dt.bfloat16),
                ("dstloc2", dstloc2, [P, BPC * NQ * K2], dt.bfloat16),
            ]:
                tl = cp.tile(shape, dtt, tag=name)
                nc.sync.dma_start(out=tl[:], in_=t[:])
                gconst[name] = tl
            ident = cp.tile([P, P], dt.float32, tag="ident")
            make_identity(nc, ident[:])
            gidx1_t = []
            for q in range(NQ):
                tl = cp.tile([P, SLEN1 // 16], dt.int16, tag=f"gidx1_{q}")
                nc.sync.dma_start(out=tl[:], in_=gidx1[q])
                gidx1_t.append(tl)
            gidx2_t = []
            for q in range(NQ):
                tl = cp.tile([P, SLEN2 // 16], dt.int16, tag=f"gidx2_{q}")
                nc.sync.dma_start(out=tl[:], in_=gidx2[q])
                gidx2_t.append(tl)

            # ---------------- phase A: T1 = (x @ W1) * dinv ------------------
            for _rep in range(int(os.environ.get("KB_REPEAT", "1"))):
                PBLK = 16          # node-blocks per DMA panel
                with (
                    tc.tile_pool(name="phA", bufs=3) as pA,
                    tc.tile_pool(name="psA", bufs=2, space="PSUM") as psA,
                ):
                    for p0 in range(0, NBINS, PBLK):
                        xt = pA.tile([P, PBLK * P], dt.bfloat16, tag="xpanel")
                        nc.sync.dma_start(out=xt[:], in_=xT[:, p0 * P : (p0 + PBLK) * P])
                        stg = pA.tile([P, PBLK * HID], dt.bfloat16, tag="stg")
                        for k in range(PBLK):
                            nb = p0 + k
                            pt = psA.tile([P, HID], dt.float32, space="PSUM", tag="pA")
                            nc.tensor.matmul(
                                out=pt[:],
                                lhsT=xt[:, k * P : (k + 1) * P],
                                rhs=gconst["W1"][:],
                                start=True,
                                stop=True,
                            )
                            nc.scalar.mul(
                                out=stg[:, k * HID : (k + 1) * HID],
                                in_=pt[:],
                                mul=gconst["dinv_node"][:, nb : nb + 1],
                            )
                        nc.sync.dma_start(
                            out=T1[p0 * P : (p0 + PBLK) * P, :].rearrange(
                                "(k p) h -> p k h", p=P
                            ),
                            in_=stg[:].rearrange("p (k h) -> p k h", h=HID),
                        )

                tc.strict_bb_all_engine_barrier()

                # ---------------- phase B: layer-1 aggregation + u2 --------------
                if stop_after == "A":
                    # debug: stop after phase A; emit a dummy y write
                    with tc.tile_pool(name="dbg", bufs=1) as dbg:
                        z = dbg.tile([P, BPC * OUT_C], dt.float32, tag="z")
                        nc.vector.memset(z[:], 0)
                        nc.sync.dma_start(
                            out=y.ap().rearrange("(b p) h -> p b h", p=P),
                            in_=z[:].rearrange("p (b h) -> p b h", h=OUT_C),
                        )
                run_b = stop_after != "A"
                if run_b:
                  with (
                    tc.tile_pool(name="phB", bufs=26) as pB,
                    tc.tile_pool(name="phBs", bufs=2) as pBs,
                    tc.tile_pool(name="phBe", bufs=4) as pBe,
                    tc.tile_pool(name="psB", bufs=2, space="PSUM") as psB,
                    tc.tile_pool(name="psBa", bufs=4, space="PSUM") as psBa,
                    tc.tile_pool(name="u2p", bufs=1) as u2pool,
                  ):
                    u2panel = u2pool.tile([P, BPC * P], dt.bfloat16, tag="u2panel")
                    nc.vector.memset(u2panel[:], 0)

                    def epi1(b, psum_agg):
                        t1 = pBe.tile([P, HID], dt.float32, tag="epi_t")
                        nc.scalar.mul(
                            out=t1[:],
                            in_=psum_agg[:],
                            mul=gconst["dinv_blk"][:, b : b + 1],
                        )
                        nc.vector.tensor_tensor(
                            out=t1[:], in0=t1[:], in1=gconst["b1x"][:], op=OP.add
                        )
                        nc.vector.tensor_scalar(
                            out=t1[:], in0=t1[:], scalar1=0.0, scalar2=None, op0=OP.max
                        )
                        ptr = psB.tile([P, P], dt.float32, space="PSUM", tag="tr")
                        nc.tensor.transpose(out=ptr[:], in_=t1[:], identity=ident[:])
                        h1t = pBe.tile([P, P], dt.float32, tag="h1t")
                        nc.vector.tensor_copy(out=h1t[:], in_=ptr[:])
                        pu = psB.tile([P, OUT_C], dt.float32, space="PSUM", tag="pu")
                        nc.tensor.matmul(
                            out=pu[:], lhsT=h1t[:], rhs=gconst["W2"][:], start=True, stop=True
                        )
                        nc.scalar.mul(
                            out=u2panel[:, b * P : b * P + OUT_C],
                            in_=pu[:],
                            mul=gconst["dinv_blk"][:, b : b + 1],
                        )

                    agg_pass(pB, pBs, psBa, gconst["iota"],
                             [T1[q * QROWS : (q + 1) * QROWS, :] for q in range(NQ)],
                             K1, gidx1_t, gconst["dstloc1"][:], HID, HID, epi1)
                    # u2 panel -> u2loc split by slot//32 (partition range per q)
                    PS = P // NQ
                    for q in range(NQ):
                        nc.sync.dma_start(
                            out=u2loc[q].rearrange("(b a) h -> a b h", a=PS),
                            in_=u2panel[q * PS : (q + 1) * PS, :].rearrange(
                                "p (b h) -> p b h", h=P
                            ),
                        )

                tc.strict_bb_all_engine_barrier()
                run_cc = stop_after not in ("A", "B")
                if run_b and stop_after == "B":
                    with tc.tile_pool(name="dbgB", bufs=1) as dbg:
                        z = dbg.tile([P, BPC * OUT_C], dt.float32, tag="zB")
                        nc.vector.memset(z[:], 0)
                        nc.sync.dma_start(
                            out=y.ap().rearrange("(b p) h -> p b h", p=P),
                            in_=z[:].rearrange("p (b h) -> p b h", h=OUT_C),
                        )
                for q in range(NQ if run_cc else 0):
                    nc.gpsimd.collective_compute(
                        "AllGather",
                        mybir.AluOpType.bypass,
                        replica_groups=[list(range(NCORES))],
                        ins=[u2loc[q]],
                        outs=[T2q[q][:]],
                    )
                tc.strict_bb_all_engine_barrier()
                run_d = stop_after not in ("A", "B", "CC")
                if run_cc and stop_after == "CC":
                    with tc.tile_pool(name="dbgC", bufs=1) as dbg:
                        z = dbg.tile([P, BPC * OUT_C], dt.float32, tag="zC")
                        nc.vector.memset(z[:], 0)
                        nc.sync.dma_start(
                            out=y.ap().rearrange("(b p) h -> p b h", p=P),
                            in_=z[:].rearrange("p (b h) -> p b h", h=OUT_C),
                        )

                # ---------------- phase D: layer-2 aggregation + FC --------------
                if run_d:
                  with (
                    tc.tile_pool(name="phD", bufs=26) as pD,
                    tc.tile_pool(name="phDs", bufs=2) as pDs,
                    tc.tile_pool(name="phDe", bufs=4) as pDe,
                    tc.tile_pool(name="psD", bufs=2, space="PSUM") as psD,
                    tc.tile_pool(name="psDa", bufs=4, space="PSUM") as psDa,
                    tc.tile_pool(name="ypl", bufs=1) as ypool,
                ):
                    ypanel = ypool.tile([P, BPC * OUT_C], dt.float32, tag="ypanel")

                    def epi2(b, psum_agg):
                        h2 = pDe.tile([P, OUT_C], dt.float32, tag="h2")
                        nc.scalar.mul(
                            out=h2[:],
                            in_=psum_agg[:],
                            mul=gconst["dinv_blk"][:, b : b + 1],
                        )
                        nc.vector.tensor_tensor(
                            out=h2[:], in0=h2[:], in1=gconst["b2x"][:], op=OP.add
                        )
                        ptr = psD.tile([OUT_C, P], dt.float32, space="PSUM", tag="tr2")
                        nc.tensor.transpose(out=ptr[:], in_=h2[:], identity=ident[:])
                        h2t = pDe.tile([OUT_C, P], dt.float32, tag="h2t")
                        nc.vector.tensor_copy(out=h2t[:], in_=ptr[:])
                        py = psD.tile([P, OUT_C], dt.float32, space="PSUM", tag="py")
                        nc.tensor.matmul(
                            out=py[:], lhsT=h2t[:], rhs=gconst["Wfc"][:], start=True, stop=True
                        )
                        nc.vector.tensor_tensor(
                            out=ypanel[:, b * OUT_C : (b + 1) * OUT_C],
                            in0=py[:],
                            in1=gconst["bfcx"][:],
                            op=OP.add,
                        )

                    agg_pass(pD, pDs, psDa, gconst["iota"],
                             [T2q[q][:] for q in range(NQ)],
                             K2, gidx2_t, gconst["dstloc2"][:], P, OUT_C, epi2,
                             nblocks=int(os.environ.get("KB_D_BLOCKS", BPC)))
                    nc.sync.dma_start(
                        out=y.ap().rearrange("(b p) h -> p b h", p=P),
                        in_=ypanel[:].rearrange("p (b h) -> p b h", h=OUT_C),
                    )

    nc.compile()
    return nc


def _make_runner(nc):
    """jit-compiled SPMD runner over 8 cores (mirrors bass2jax.run_bass_via_pjrt
    but reusable across calls so executions can be timed warm)."""
    import jax
    import numpy as np
    from jax.sharding import Mesh, PartitionSpec
    from jax.experimental.shard_map import shard_map
    import concourse.mybir as mybir
    from concourse import bass2jax

    bass2jax.install_neuronx_cc_hook()
    partition_name = nc.partition_id_tensor.name if nc.partition_id_tensor else None
    in_names, out_names, out_avals, zero_outs = [], [], [], []
    for alloc in nc.m.functions[0].allocations:
        if not isinstance(alloc, mybir.MemoryLocationSet):
            continue
        name = alloc.memorylocations[0].name
        if alloc.kind == "ExternalInput":
            if name != partition_name:
                in_names.append(name)
        elif alloc.kind == "ExternalOutput":
            out_names.append(name)
            shape = tuple(alloc.tensor_shape)
            dtype = mybir.dt.np(alloc.dtype)
            out_avals.append(jax.core.ShapedArray(shape, dtype))
            zero_outs.append(np.zeros(shape, dtype))
    n_params = len(in_names)
    all_in_names = list(in_names) + list(out_names)
    if partition_name is not None:
        all_in_names.append(partition_name)

    def _body(*args):
        operands = list(args)
        if partition_name is not None:
            operands.append(bass2jax.partition_id_tensor())
        outs = bass2jax._bass_exec_p.bind(
            *operands,
            out_avals=tuple(out_avals),
            in_names=tuple(all_in_names),
            out_names=tuple(out_names),
            lowering_input_output_aliases=(),
            sim_require_finite=True,
            sim_require_nnan=True,
            nc=nc,
        )
        return tuple(outs)

    devices = jax.devices()[:NCORES]
    mesh = Mesh(np.asarray(devices), ("core",))
    in_specs = (PartitionSpec("core"),) * (n_params + len(out_names))
    out_specs = (PartitionSpec("core"),) * len(out_names)
    fn = jax.jit(
        shard_map(_body, mesh=mesh, in_specs=in_specs, out_specs=out_specs,
                  check_rep=False),
        keep_unused=True,
    )
    return fn, in_names, out_names, zero_outs, mesh


def kernel(x, edge_index, W1, b1, W2, b2, Wfc, bfc, _trace=False, _bench=True):
    import time as _time
    import jax
    from jax.sharding import NamedSharding, PartitionSpec

    import os as _os
    K1, K2, in_maps, perm_pos = _preprocess(x, edge_index, W1, b1, W2, b2, Wfc, bfc)
    key = (K1, K2, _os.environ.get("KB_REPEAT", "1"),
           _os.environ.get("KB_STOP_AFTER", ""), _os.environ.get("KB_D_BLOCKS", ""))
    if key not in _kernel_cache:
        nc = _build(K1, K2)
        _kernel_cache[key] = (nc, _make_runner(nc))
    nc, (fn, in_names, out_names, zero_outs, mesh) = _kernel_cache[key]

    sh = NamedSharding(mesh, PartitionSpec("core"))
    concat_in = [
        np.concatenate([np.asarray(in_maps[c][nm]) for c in range(NCORES)], axis=0)
        for nm in in_names
    ]
    concat_zeros = [
        np.zeros((NCORES * z.shape[0], *z.shape[1:]), z.dtype) for z in zero_outs
    ]
    dev_in = [jax.device_put(a, sh) for a in concat_in + concat_zeros]
    out_arrs = fn(*dev_in)
    jax.block_until_ready(out_arrs)

    if _bench:
        times = []
        for _ in range(5):
            t0 = _time.perf_counter()
            out_arrs = fn(*dev_in)
            jax.block_until_ready(out_arrs)
            times.append(_time.perf_counter() - t0)
        kernel._last_times = times
        kernel._last_exec_time_ns = int(min(times) * 1e9)
    else:
        kernel._last_exec_time_ns = None
    if not hasattr(kernel, "_runners"):
        kernel._runners = {}
    kernel._runners[_os.environ.get("KB_REPEAT", "1")] = (fn, dev_in)

    outs = {nm: np.asarray(out_arrs[i]) for i, nm in enumerate(out_names)}
    Y = outs["y"].reshape(NCORES, SHARD, OUT_C).reshape(NCORES * SHARD, OUT_C)
    return Y[perm_pos[:N]].astype(np.float32)

